# revision 4
# baseline (speedup 1.0000x reference)
"""Causal self-attention (B=4, T=2048, D=1024, H=16, hd=64) on 8 TRN2 NeuronCores.

Sharding: core c handles batch b = c % 4 and head-half = c // 4 (8 heads each).
Each core computes, for its (batch, 8 heads):
    qkv projection -> causal attention -> partial output projection (yT).
Host gathers: y[b] = (yT[core b] + yT[core b+4]).T + b_proj.

Device design (per core):
  - transposed layouts: xT [D, T], qT/kT [hd-stacked, T], output yT [D, T]
  - S computed as S^T [k, q] tiles: lhsT = kT-slice, rhs = qT-slice; two heads
    row-packed on the PE at tile_position (0,0)/(64,0) (contraction = hd = 64)
  - softmax without max-subtraction (logits are small); exp on ScalarE with
    the 1/sqrt(hd) scale fused; causal masking via gpsimd affine_select on
    diagonal tiles only
  - V' = [V | 1] trick: wv gets a zero 65th column per head and bv' a 1.0, so
    each PV matmul (M=65, fp32r-legal dst partition 0) also produces the
    softmax denominator in row 64
  - normalize: DVE reciprocal (partition 64 -> 0/32), DMA partition-broadcast,
    DVE multiply into the proj input layout
  - all matmuls float32r (stored fp32 bits, full PE rate at N >= 256)
"""

import os
import sys
from contextlib import ExitStack

import numpy as np

sys.path.insert(0, "/opt/trn_rl_repo")

import concourse.bass as bass  # noqa: E402
import concourse.tile as tile  # noqa: E402
from concourse import bacc, mybir  # noqa: E402

f32 = mybir.dt.float32
f32r = mybir.dt.float32r
EXP = mybir.ActivationFunctionType.Exp

B, T, D = 4, 2048, 1024
H, HD = 16, 64
HDP = HD + 1       # 65: head dim + ones column
HPC = 8            # heads per core
NP = 4             # head pairs per core
NCORES = 8
TCH = 256          # phase-1 t-chunk width
NTCH = T // TCH    # 8
QCH = 512          # attention q-chunk width
NQCH = T // QCH    # 4
NKT = T // 128     # 16 k-tiles
VW = HPC * HDP     # 520: V' width
VH = VW // 2       # 260: V' half width (one matmul's N)


def _mm(nc, out, lhsT, rhs, **kw):
    nc.tensor.matmul(out, lhsT.bitcast(f32r), rhs.bitcast(f32r), **kw)


def build_program():
    nc = bacc.Bacc("TRN2", target_bir_lowering=False, debug=False)

    xT = nc.dram_tensor("xT", [D, T], f32r, kind="ExternalInput").ap()
    wqk = nc.dram_tensor("wqk", [D, 2 * HPC * HD], f32r, kind="ExternalInput").ap()
    wv = nc.dram_tensor("wv", [D, VW], f32r, kind="ExternalInput").ap()
    wp = nc.dram_tensor("wp", [HPC * HD, D], f32r, kind="ExternalInput").ap()
    bqk = nc.dram_tensor("bqk", [2 * HPC * HD, 1], f32, kind="ExternalInput").ap()
    bv = nc.dram_tensor("bv", [128, VW], f32, kind="ExternalInput").ap()
    yT = nc.dram_tensor("yT", [D, T], f32, kind="ExternalOutput").ap()

    with tile.TileContext(nc) as tc:
        with ExitStack() as ctx:
            _build(ctx, tc, xT, wqk, wv, wp, bqk, bv, yT)
    nc.compile()
    return nc


def _build(ctx, tc, xT, wqk, wv, wp, bqk, bv, yT):
    nc = tc.nc

    persist = ctx.enter_context(tc.tile_pool(name="persist", bufs=1))
    wqk_pool = ctx.enter_context(tc.tile_pool(name="wqk_pool", bufs=4))
    w16 = ctx.enter_context(tc.tile_pool(name="w16", bufs=1))
    xc_pool = ctx.enter_context(tc.tile_pool(name="xc_pool", bufs=1))
    es_pool = ctx.enter_context(tc.tile_pool(name="es_pool", bufs=2))
    pin_pool = ctx.enter_context(tc.tile_pool(name="pin_pool", bufs=1))
    small = ctx.enter_context(tc.tile_pool(name="small", bufs=6))

    p1ps = ctx.enter_context(tc.tile_pool(name="p1ps", bufs=2, space="PSUM"))
    s_ps_pool = ctx.enter_context(tc.tile_pool(name="s_ps_pool", bufs=3, space="PSUM"))
    pv_ps_pool = ctx.enter_context(tc.tile_pool(name="pv_ps_pool", bufs=2, space="PSUM"))
    y_ps_pool = ctx.enter_context(tc.tile_pool(name="y_ps_pool", bufs=1, space="PSUM"))

    # ---- persistent tensors ----
    qT = persist.tile([128, NP, T], f32r, tag="qT")     # [2 heads x 64 dims, pair, t]
    kT = persist.tile([128, NP, T], f32r, tag="kT")
    V = persist.tile([128, NKT, VW], f32r, tag="V")     # [t in tile, k-tile, h*65+d]

    # ---- constants / weights ----
    wqk_sb = []
    for s in range(4):
        w = wqk_pool.tile([128, 2, 2 * HPC * HD], f32r, name=f"wqk_sb{s}", tag="wqk")
        nc.sync.dma_start(
            out=w, in_=wqk[2 * s * 128:(2 * s + 2) * 128, :].rearrange(
                "(i p) m -> p i m", p=128))
        wqk_sb.append(w)
    wv_sb = w16.tile([128, 8, VW], f32r, tag="wv_sb")
    nc.sync.dma_start(out=wv_sb, in_=wv.rearrange("(d p) n -> p d n", p=128))

    bqk_sb = small.tile([128, 8], f32, tag="bqk_sb", bufs=1)
    nc.sync.dma_start(out=bqk_sb, in_=bqk.rearrange("(m p) o -> p (m o)", p=128))
    bv_sb = small.tile([128, VW], f32, tag="bv_sb", bufs=1)
    nc.sync.dma_start(out=bv_sb, in_=bv)
    zreg = nc.gpsimd.to_reg(0.0)

    wp_sb = None  # loaded lazily after phase-1 starts

    # ================= phase 1: qkv projection (per t-chunk) =================
    def p1_chunk(tc_i):
        xc = xc_pool.tile([128, 8, TCH], f32r, name=f"xc{tc_i}", tag="xc")
        nc.sync.dma_start(
            out=xc,
            in_=xT.rearrange("(d p) t -> p d t", p=128)[:, :, tc_i * TCH:(tc_i + 1) * TCH])
        # q^T / k^T: out m-tile rows, t cols
        for mt in range(8):
            qk_ps = p1ps.tile([128, TCH], f32, name=f"qk_ps_{tc_i}_{mt}", tag="p1")
            for dt in range(8):
                _mm(nc, qk_ps, wqk_sb[dt // 2][:, dt % 2, mt * 128:(mt + 1) * 128],
                    xc[:, dt, :], start=(dt == 0), stop=(dt == 7))
            dest = qT if mt < 4 else kT
            nc.vector.tensor_scalar_add(
                dest[:, mt % 4, tc_i * TCH:(tc_i + 1) * TCH], qk_ps, bqk_sb[:, mt:mt + 1])
        # V': natural layout [t, h*65+d], two N=260 matmul groups per t-tile
        for tt in range(TCH // 128):
            t_idx = tc_i * (TCH // 128) + tt
            for g in range(2):
                v_ps = p1ps.tile([128, 512], f32, name=f"v_ps_{tc_i}_{tt}_{g}", tag="p1")
                for dt in range(8):
                    _mm(nc, v_ps[:, 0:VH], xc[:, dt, tt * 128:(tt + 1) * 128],
                        wv_sb[:, dt, g * VH:(g + 1) * VH],
                        start=(dt == 0), stop=(dt == 7))
                nc.vector.tensor_tensor(
                    out=V[:, t_idx, g * VH:(g + 1) * VH], in0=v_ps[:, 0:VH],
                    in1=bv_sb[:, g * VH:(g + 1) * VH], op=mybir.AluOpType.add)

    # ================= phase 2: attention for q-chunk j =================
    def attn_chunk(j):
        q0 = j * QCH
        for p in range(NP):
            pvA = pv_ps_pool.tile([128, QCH], f32, name=f"pvA_{j}_{p}", tag="pv")
            pvB = pv_ps_pool.tile([128, QCH], f32, name=f"pvB_{j}_{p}", tag="pv")
            nkt = 4 * (j + 1)
            last = nkt - 1
            for kt in range(nkt):
                sA = s_ps_pool.tile([128, QCH], f32, name=f"sA_{j}_{p}_{kt}", tag="s")
                sB = s_ps_pool.tile([128, QCH], f32, name=f"sB_{j}_{p}_{kt}", tag="s")
                _mm(nc, sA, kT[0:64, p, kt * 128:(kt + 1) * 128],
                    qT[0:64, p, q0:q0 + QCH],
                    start=True, stop=True, tile_position=(0, 0))
                _mm(nc, sB, kT[64:128, p, kt * 128:(kt + 1) * 128],
                    qT[64:128, p, q0:q0 + QCH],
                    start=True, stop=True, tile_position=(64, 0))
                e = es_pool.tile([128, 2 * QCH], f32r, name=f"e_{j}_{p}_{kt}", tag="e")
                eA = e[:, 0:QCH]
                eB = e[:, QCH:2 * QCH]
                nc.scalar.activation(eA, sA, EXP, scale=0.125)
                nc.scalar.activation(eB, sB, EXP, scale=0.125)
                if kt >= 4 * j:  # diagonal tile: keep ql >= kl + 128*o
                    o = kt - 4 * j
                    for sl in (eA, eB):
                        nc.gpsimd.affine_select(
                            sl, sl, pattern=[[1, QCH]],
                            compare_op=mybir.AluOpType.is_ge, fill=zreg,
                            base=-(128 * o), channel_multiplier=-1)
                hA, hB = 2 * p, 2 * p + 1
                _mm(nc, pvA[0:HDP, :], V[:, kt, hA * HDP:(hA + 1) * HDP], eA,
                    start=(kt == 0), stop=(kt == last))
                _mm(nc, pvB[0:HDP, :], V[:, kt, hB * HDP:(hB + 1) * HDP], eB,
                    start=(kt == 0), stop=(kt == last))
            # normalize: proj_in[:, p, :] = U * (1/sums), U rows 0:64, sums row 64
            stage = small.tile([33, QCH], f32, name=f"stage_{j}_{p}", tag="st", bufs=2)
            nc.vector.reciprocal(stage[0:1, :], pvA[64:65, :])
            nc.vector.reciprocal(stage[32:33, :], pvB[64:65, :])
            bcast = small.tile([128, QCH], f32, name=f"bcast_{j}_{p}", tag="bc", bufs=2)
            ss = stage[0:33:32, :]
            nc.sync.dma_start(
                out=bcast,
                in_=bass.AP(tensor=ss.tensor, offset=ss.offset,
                            ap=[ss.ap[0], [0, 64], ss.ap[1]]))
            nc.vector.tensor_tensor(out=proj_in[0:64, p, :], in0=pvA[0:64, :],
                                    in1=bcast[0:64, :], op=mybir.AluOpType.mult)
            nc.vector.tensor_tensor(out=proj_in[64:128, p, :], in0=pvB[0:64, :],
                                    in1=bcast[64:128, :], op=mybir.AluOpType.mult)

    # ================= phase 3: output projection for q-chunk j =================
    def proj_chunk(j):
        for mt in range(8):
            y_ps = y_ps_pool.tile([128, QCH], f32, name=f"y_{j}_{mt}", tag="y")
            for p in range(NP):
                _mm(nc, y_ps, wp_sb[:, p, mt * 128:(mt + 1) * 128], proj_in[:, p, :],
                    start=(p == 0), stop=(p == NP - 1))
            y_sb = small.tile([128, QCH], f32, name=f"ysb_{j}_{mt}", tag="ysb", bufs=2)
            nc.vector.tensor_copy(out=y_sb, in_=y_ps)
            nc.sync.dma_start(
                out=yT[mt * 128:(mt + 1) * 128, j * QCH:(j + 1) * QCH], in_=y_sb)

    # ================= emission: interleave =================
    proj_in = pin_pool.tile([128, NP, QCH], f32r, tag="pin")
    for tc_i in range(NTCH):
        p1_chunk(tc_i)
        if tc_i == 1:
            wp_sb = w16.tile([128, NP, D], f32r, tag="wp_sb")
            nc.sync.dma_start(out=wp_sb, in_=wp.rearrange("(k p) m -> p k m", p=128))
        if tc_i % 2 == 1:
            j = tc_i // 2
            attn_chunk(j)
            proj_chunk(j)


# ======================= host side =======================

_NC_CACHE = None
LAST_RESULT = None


def _get_program():
    global _NC_CACHE
    if _NC_CACHE is None:
        _NC_CACHE = build_program()
    return _NC_CACHE


def shard_inputs(x, w_qkv, b_qkv, w_proj):
    x = np.asarray(x, dtype=np.float32)
    w_qkv = np.asarray(w_qkv, dtype=np.float32)
    b_qkv = np.asarray(b_qkv, dtype=np.float32)
    w_proj = np.asarray(w_proj, dtype=np.float32)
    in_maps = []
    for c in range(NCORES):
        b = c % B
        half = c // B
        hs = half * (HPC * HD)  # 512
        wq = w_qkv[:, 0 * D + hs:0 * D + hs + HPC * HD]
        wk = w_qkv[:, 1 * D + hs:1 * D + hs + HPC * HD]
        wv_ = w_qkv[:, 2 * D + hs:2 * D + hs + HPC * HD]
        bq = b_qkv[0 * D + hs:0 * D + hs + HPC * HD]
        bk = b_qkv[1 * D + hs:1 * D + hs + HPC * HD]
        bv_ = b_qkv[2 * D + hs:2 * D + hs + HPC * HD]
        # V' = [V | 1]: wv gets a zero 65th column per head; bv' a 1.0 there
        wvp = np.zeros((D, VW), dtype=np.float32)
        bvp = np.zeros((VW,), dtype=np.float32)
        for h in range(HPC):
            wvp[:, h * HDP:h * HDP + HD] = wv_[:, h * HD:(h + 1) * HD]
            bvp[h * HDP:h * HDP + HD] = bv_[h * HD:(h + 1) * HD]
            bvp[h * HDP + HD] = 1.0
        in_maps.append({
            "xT": np.ascontiguousarray(x[b].T),
            "wqk": np.ascontiguousarray(np.concatenate([wq, wk], axis=1)),
            "wv": wvp,
            "wp": np.ascontiguousarray(w_proj[hs:hs + HPC * HD, :]),
            "bqk": np.ascontiguousarray(np.concatenate([bq, bk])[:, None]),
            "bv": np.ascontiguousarray(np.broadcast_to(bvp[None, :], (128, VW))),
        })
    return in_maps


def kernel(x, w_qkv, b_qkv, w_proj, b_proj):
    global LAST_RESULT
    from concourse.bass_utils import run_bass_kernel_spmd

    nc = _get_program()
    in_maps = shard_inputs(x, w_qkv, b_qkv, w_proj)
    res = run_bass_kernel_spmd(nc, in_maps, list(range(NCORES)))
    LAST_RESULT = res
    b_proj = np.asarray(b_proj, dtype=np.float32)
    y = np.empty((B, T, D), dtype=np.float32)
    for b in range(B):
        yTfull = res.results[b]["yT"] + res.results[b + B]["yT"]
        y[b] = yTfull.T + b_proj[None, :]
    return y
